# revision 5
# baseline (speedup 1.0000x reference)
"""Trainium2 Bass kernel for nn_Decoder_128849019075 (single-step LSTM decoder
with attention + vocab projection + log_softmax), SPMD across 8 NeuronCores.

Sharding: LSTM gate columns / attention DH-chunk / vocab columns are all split
8 ways (128-wide DH chunks, 4000-wide vocab chunks). Activations live
feature-major ("T" = transposed, [feature, batch]) on device. Cross-core:
AG(h0n) -> AG(h1n) -> AR(scores) -> AG(ctx) -> AG(softmax stats).

Key algebraic rewrite vs the reference: scores[s,b] = sum_e enc[s,b,e]*q[b,e]
with q = ro @ attn_W (instead of materializing energy = enc @ attn_W.T), plus
the constant term ro . attn_b added after the cross-core reduce.
"""
import sys

for _p in ("/opt/trn_rl_repo",):
    if _p not in sys.path:
        sys.path.insert(0, _p)

import numpy as np

import concourse.bacc as bacc
import concourse.bass as bass
import concourse.tile as tile
import concourse.mybir as mybir
from concourse import bass_utils
from concourse.masks import make_identity

F32 = mybir.dt.float32
I32 = mybir.dt.int32
AF = mybir.ActivationFunctionType
OP = mybir.AluOpType

H, DH, V, B, S, NC = 512, 1024, 32000, 64, 128, 8
DHC = DH // NC      # 128  per-core DH / attention chunk
VC = V // NC        # 4000 per-core vocab chunk
NV = 8              # vocab sub-chunks per core
VB = VC // NV       # 500  (<=512, one PSUM bank)
NVA = 6             # sub-chunks accumulated in phase A (k 0..7)
KE0 = 3 * H // 128  # 12 k-tiles for layer-0 x
KH = DH // 128      # 8 k-tiles for DH-sized contractions


def _build():
    nc = bacc.Bacc("TRN2", target_bir_lowering=False, debug=False,
                   enable_asserts=True, num_devices=NC)

    def din(name, shape, dtype=F32):
        return nc.dram_tensor(name, shape, dtype, kind="ExternalInput")

    def dout(name, shape, dtype=F32):
        return nc.dram_tensor(name, shape, dtype, kind="ExternalOutput")

    idx_d = din("idx", [B, 1], I32)
    emb_d = din("emb", [V, H])
    ctxT_d = din("ctxT", [DH, B])
    h0T0_d = din("h0T0", [DH, B])
    h0T1_d = din("h0T1", [DH, B])
    c0T0_d = din("c0T0", [DHC, B])
    c0T1_d = din("c0T1", [DHC, B])
    wih0_d = din("wih0t", [3 * H, 512])
    whh0_d = din("whh0t", [DH, 512])
    wih1_d = din("wih1t", [DH, 512])
    whh1_d = din("whh1t", [DH, 512])
    bias0_d = din("bias0", [DHC, 4])
    bias1_d = din("bias1", [DHC, 4])
    attnw_d = din("attnw", [DH, DHC])
    attnbt_d = din("attnbt", [DHC, NC])
    outwt_d = din("outwt", [4 * H, VC])
    outb_d = din("outb", [1, VC])
    enc_d = din("enc", [S, B, DHC])

    logp_d = dout("logp", [B, VC])
    ctxf_d = dout("ctx_full", [DH, B])
    h0n_d = dout("h0n", [DHC, B])
    h1n_d = dout("h1n", [DHC, B])
    c0n_d = dout("c0n", [DHC, B])
    c1n_d = dout("c1n", [DHC, B])
    logw_d = dout("logw", [S, B])

    with tile.TileContext(nc) as tc:
        with (
            tc.tile_pool(name="persist", bufs=1) as pp,
            tc.tile_pool(name="stream", bufs=3) as sp,
            tc.tile_pool(name="big", bufs=2) as bigp,
            tc.tile_pool(name="work", bufs=1) as wp,
            tc.tile_pool(name="psA", bufs=1, space="PSUM") as psA,
            tc.tile_pool(name="psB", bufs=1, space="PSUM") as psB,
            tc.tile_pool(name="dram", bufs=1, space="DRAM") as dp,
        ):
            # ---------- constants / tiny inputs ----------
            ident = pp.tile([128, 128], F32, tag="ident")
            make_identity(nc, ident[:])
            ones1 = pp.tile([1, B], F32, tag="ones1")
            nc.vector.memset(ones1[:], 1.0)

            idx_t = pp.tile([B, 1], I32, tag="idx")
            nc.scalar.dma_start(idx_t[:], idx_d[:, :])
            bias0_t = pp.tile([DHC, 4], F32, tag="bias0")
            nc.scalar.dma_start(bias0_t[:], bias0_d[:, :])
            bias1_t = pp.tile([DHC, 4], F32, tag="bias1")
            nc.scalar.dma_start(bias1_t[:], bias1_d[:, :])
            attnbt_t = pp.tile([DHC, NC], F32, tag="attnbt")
            nc.scalar.dma_start(attnbt_t[:], attnbt_d[:, :])
            outb_t = pp.tile([1, VC], F32, tag="outb")
            nc.scalar.dma_start(outb_t[:], outb_d[:, :])

            # persistent activation tiles, feature-major [128, r, B]
            def load_T(dram, name):
                t = pp.tile([128, KH, B], F32, tag=name)
                nc.scalar.dma_start(
                    t[:], dram[:, :].rearrange("(r p) b -> p r b", p=128))
                return t

            ctxT_t = load_T(ctxT_d, "ctxT")
            h0T0_t = load_T(h0T0_d, "h0T0")
            h0T1_t = load_T(h0T1_d, "h0T1")
            c0T0_t = pp.tile([DHC, B], F32, tag="c0T0")
            nc.scalar.dma_start(c0T0_t[:], c0T0_d[:, :])
            c0T1_t = pp.tile([DHC, B], F32, tag="c0T1")
            nc.scalar.dma_start(c0T1_t[:], c0T1_d[:, :])

            # ---------- embedding gather + transpose ----------
            emb_sb = pp.tile([B, H], F32, tag="embsb")
            nc.gpsimd.indirect_dma_start(
                out=emb_sb[:], out_offset=None, in_=emb_d[:, :],
                in_offset=bass.IndirectOffsetOnAxis(ap=idx_t[:, :1], axis=0),
            )
            embT_t = pp.tile([128, 4, B], F32, tag="embT")
            for c in range(4):
                tp = psA.tile([128, B], F32, tag=f"pa{c % 2}")
                nc.tensor.transpose(
                    tp[:], emb_sb[:, c * 128:(c + 1) * 128], ident[:B, :B])
                nc.vector.tensor_copy(embT_t[:, c, :], tp[:])

            # ---------- LSTM weights (streamed) ----------
            def wtile(dram):
                t = sp.tile([128, KH, 512], F32, tag="stream")
                nc.sync.dma_start(
                    t[:], dram[:, :].rearrange("(r p) g -> p r g", p=128))
                return t

            wih0a = sp.tile([128, 6, 512], F32, tag="stream")
            nc.sync.dma_start(
                wih0a[:], wih0_d[0:768, :].rearrange("(r p) g -> p r g", p=128))
            wih0b = sp.tile([128, 6, 512], F32, tag="stream")
            nc.sync.dma_start(
                wih0b[:], wih0_d[768:1536, :].rearrange("(r p) g -> p r g", p=128))
            whh0 = wtile(whh0_d)

            # ---------- layer 0 ----------
            def xT_tile(kt):
                if kt < 4:
                    return embT_t[:, kt, :]
                return ctxT_t[:, kt - 4, :]

            def lstm_cell(wih_tiles, whh_t, hT_t, bias_t, c0T_t, n_x, xf,
                          h_out_t, c_out_t):
                """wih_tiles(kt) -> [128,512] lhsT source for x-side k-tile kt;
                xf(kt) -> [128,B] rhs. whh_t [128,KH,512], hT_t [128,KH,B]."""
                gates = [psB.tile([DHC, B], F32, name=f"gates{g}", tag=f"pb{g}")
                         for g in range(4)]
                nk = n_x + KH
                for kt in range(nk):
                    if kt < n_x:
                        wsrc, rhs = wih_tiles(kt), xf(kt)
                    else:
                        wsrc, rhs = whh_t[:, kt - n_x, :], hT_t[:, kt - n_x, :]
                    for g in range(4):
                        nc.tensor.matmul(
                            gates[g][:], wsrc[:, g * 128:(g + 1) * 128], rhs,
                            start=(kt == 0), stop=(kt == nk - 1))
                acts = []
                for g, fn in enumerate((AF.Sigmoid, AF.Sigmoid, AF.Tanh,
                                        AF.Sigmoid)):
                    a = wp.tile([DHC, B], F32, tag=f"act{g}")
                    nc.scalar.activation(a[:], gates[g][:], fn,
                                         bias=bias_t[:, g:g + 1])
                    acts.append(a)
                ig = wp.tile([DHC, B], F32, tag="ig")
                nc.vector.tensor_mul(ig[:], acts[0][:], acts[2][:])
                fc = wp.tile([DHC, B], F32, tag="fc")
                nc.vector.tensor_mul(fc[:], acts[1][:], c0T_t[:])
                nc.vector.tensor_add(c_out_t[:], ig[:], fc[:])
                tc_t = wp.tile([DHC, B], F32, tag="tanhc")
                nc.scalar.activation(tc_t[:], c_out_t[:], AF.Tanh)
                nc.vector.tensor_mul(h_out_t[:], acts[3][:], tc_t[:])

            h0n_t = pp.tile([DHC, B], F32, tag="h0n")
            c0n_t = pp.tile([DHC, B], F32, tag="c0n")
            lstm_cell(lambda kt: (wih0a if kt < 6 else wih0b)[:, kt % 6, :],
                      whh0, h0T0_t, bias0_t, c0T0_t, KE0, xT_tile,
                      h0n_t, c0n_t)
            nc.scalar.dma_start(h0n_d[:, :], h0n_t[:])
            nc.scalar.dma_start(c0n_d[:, :], c0n_t[:])

            # ---------- AG h0n ----------
            b_h0_in = dp.tile([DHC, B], F32, tag="b_h0_in")
            b_h0_out = dp.tile([NC * DHC, B], F32, tag="b_h0_out")
            nc.scalar.dma_start(b_h0_in[:], h0n_t[:])
            nc.gpsimd.collective_compute(
                "AllGather", OP.bypass, replica_groups=[list(range(NC))],
                ins=[b_h0_in.opt()], outs=[b_h0_out.opt()])
            h0nF_t = pp.tile([128, KH, B], F32, tag="h0nF")
            nc.scalar.dma_start(
                h0nF_t[:], b_h0_out[:, :].rearrange("(r p) b -> p r b", p=128))

            # ---------- layer 1 ----------
            wih1 = wtile(wih1_d)
            whh1 = wtile(whh1_d)
            h1n_t = pp.tile([DHC, B], F32, tag="h1n")
            c1n_t = pp.tile([DHC, B], F32, tag="c1n")
            lstm_cell(lambda kt: wih1[:, kt, :], whh1, h0T1_t, bias1_t, c0T1_t,
                      KH, lambda kt: h0nF_t[:, kt, :], h1n_t, c1n_t)
            nc.scalar.dma_start(h1n_d[:, :], h1n_t[:])
            nc.scalar.dma_start(c1n_d[:, :], c1n_t[:])

            # ---------- AG h1n -> ro ----------
            b_h1_in = dp.tile([DHC, B], F32, tag="b_h1_in")
            b_h1_out = dp.tile([NC * DHC, B], F32, tag="b_h1_out")
            nc.scalar.dma_start(b_h1_in[:], h1n_t[:])
            nc.gpsimd.collective_compute(
                "AllGather", OP.bypass, replica_groups=[list(range(NC))],
                ins=[b_h1_in.opt()], outs=[b_h1_out.opt()])
            roT_t = pp.tile([128, KH, B], F32, tag="roT")
            nc.scalar.dma_start(
                roT_t[:], b_h1_out[:, :].rearrange("(r p) b -> p r b", p=128))

            # ---------- attention: q, ab, scores ----------
            attnw_t = sp.tile([128, KH, DHC], F32, tag="stream")
            nc.sync.dma_start(
                attnw_t[:], attnw_d[:, :].rearrange("(r p) e -> p r e", p=128))
            enc_t = bigp.tile([S, B * DHC], F32, tag="big")
            nc.sync.dma_start(enc_t[:],
                              enc_d[:, :, :].rearrange("s b e -> s (b e)"))

            q_ps = psA.tile([B, DHC], F32, tag="pa0")
            ab_ps = psA.tile([1, B], F32, tag="pa1")
            for kt in range(KH):
                nc.tensor.matmul(q_ps[:], roT_t[:, kt, :], attnw_t[:, kt, :],
                                 start=(kt == 0), stop=(kt == KH - 1))
            for kt in range(KH):
                nc.tensor.matmul(ab_ps[:], attnbt_t[:, kt:kt + 1],
                                 roT_t[:, kt, :],
                                 start=(kt == 0), stop=(kt == KH - 1))
            q_sb = wp.tile([B, DHC], F32, tag="qsb")
            nc.vector.tensor_copy(q_sb[:], q_ps[:])
            ab_sb = wp.tile([1, B], F32, tag="absb")
            nc.vector.tensor_copy(ab_sb[:], ab_ps[:])

            qrow = pp.tile([1, B * DHC], F32, tag="qrow")
            nc.scalar.dma_start(qrow[:], q_sb[:])
            qbig = bigp.tile([S, B * DHC], F32, tag="big")
            nc.gpsimd.partition_broadcast(qbig[:], qrow[:])

            scores_t = pp.tile([S, B], F32, tag="scores")
            for b in range(B):
                bs = slice(b * DHC, (b + 1) * DHC)
                nc.vector.scalar_tensor_tensor(
                    out=qbig[:, bs], in0=enc_t[:, bs], scalar=1.0,
                    in1=qbig[:, bs], op0=OP.mult, op1=OP.mult,
                    accum_out=scores_t[:, b:b + 1])

            # ---------- AR scores ----------
            b_sc_in = dp.tile([S, B], F32, tag="b_sc_in")
            b_sc_out = dp.tile([S, B], F32, tag="b_sc_out")
            nc.scalar.dma_start(b_sc_in[:], scores_t[:])
            nc.gpsimd.collective_compute(
                "AllReduce", OP.add, replica_groups=[list(range(NC))],
                ins=[b_sc_in.opt()], outs=[b_sc_out.opt()])
            scsum_t = pp.tile([S, B], F32, tag="scsum")
            nc.scalar.dma_start(scsum_t[:], b_sc_out[:, :])

            # add ab (broadcast along s) then log_softmax over b (free dim)
            ab_big = wp.tile([S, B], F32, tag="abbig")
            nc.gpsimd.partition_broadcast(ab_big[:], ab_sb[:])
            nc.vector.tensor_add(scsum_t[:], scsum_t[:], ab_big[:])
            sm_m = wp.tile([S, 1], F32, tag="smm")
            nc.vector.reduce_max(out=sm_m[:], in_=scsum_t[:],
                                 axis=mybir.AxisListType.X)
            nc.vector.tensor_scalar_mul(sm_m[:], sm_m[:], -1.0)
            sm_s = wp.tile([S, 1], F32, tag="sms")
            nc.scalar.activation(ab_big[:], scsum_t[:], AF.Exp,
                                 bias=sm_m[:], accum_out=sm_s[:])
            nc.scalar.activation(sm_s[:], sm_s[:], AF.Ln)
            # sm_m holds -max; lse = max + ln(sum); -lse = sm_m - ln(sum)
            nc.vector.tensor_sub(sm_s[:], sm_m[:], sm_s[:])
            logw_t = pp.tile([S, B], F32, tag="logw")
            nc.vector.tensor_scalar_add(logw_t[:], scsum_t[:], sm_s[:])
            nc.scalar.dma_start(logw_d[:, :], logw_t[:])

            # ---------- ctx per-b matvecs ----------
            ctx_ps = psA.tile([DHC, B], F32, tag="pa0")
            for b in range(B):
                bs = slice(b * DHC, (b + 1) * DHC)
                nc.tensor.matmul(ctx_ps[:, b:b + 1], enc_t[:, bs],
                                 logw_t[:, b:b + 1], start=True, stop=True)
            ctx_sb = pp.tile([DHC, B], F32, tag="ctxsb")
            nc.vector.tensor_copy(ctx_sb[:], ctx_ps[:])

            # ---------- AG ctx ----------
            b_cx_in = dp.tile([DHC, B], F32, tag="b_cx_in")
            b_cx_out = dp.tile([NC * DHC, B], F32, tag="b_cx_out")
            nc.scalar.dma_start(b_cx_in[:], ctx_sb[:])
            nc.gpsimd.collective_compute(
                "AllGather", OP.bypass, replica_groups=[list(range(NC))],
                ins=[b_cx_in.opt()], outs=[b_cx_out.opt()])
            ctxF_t = pp.tile([128, KH, B], F32, tag="ctxF")
            nc.scalar.dma_start(
                ctxF_t[:], b_cx_out[:, :].rearrange("(r p) b -> p r b", p=128))
            nc.scalar.dma_start(ctxf_d[:, :], b_cx_out[:, :])

            # ---------- vocab projection ----------
            # actT k-tiles: 0..7 -> roT, 8..15 -> ctxF
            def act_tile(kt):
                return roT_t[:, kt, :] if kt < KH else ctxF_t[:, kt - KH, :]

            # rows 0:64 = logits, 64:128 = scratch (exp / final output)
            logits_t = pp.tile([128, VC], F32, tag="logits")
            lg = logits_t[0:B, :]
            scratch = logits_t[B:2 * B, :]

            # phase A: k 0..7 (ro half) for v-chunks 0..NVA-1, k-outer so each
            # streamed outwt tile is fully consumed before the next arrives.
            psA_l = [psB.tile([B, VB], F32, name=f"psAl{j}", tag=f"pb{j}")
                     for j in range(NVA)]
            for kt in range(KH):
                t = sp.tile([128, VC], F32, tag="stream")
                nc.sync.dma_start(t[:], outwt_d[kt * 128:(kt + 1) * 128, :])
                for j in range(NVA):
                    nc.tensor.matmul(
                        psA_l[j][:], act_tile(kt),
                        t[:, j * VB:(j + 1) * VB],
                        start=(kt == 0), stop=False)
            for j in range(NVA):
                vs = slice(j * VB, (j + 1) * VB)
                nc.tensor.matmul(psA_l[j][:], ones1[:, :], outb_t[:, vs],
                                 start=False, stop=True)
                nc.vector.tensor_copy(lg[:, vs], psA_l[j][:])

            # phase B: k 8..15 for all v-chunks; v-chunks NVA..7 also get
            # k 0..7 from a re-read of the outwt columns (full tiles are gone).
            psB_l = [psB.tile([B, VB], F32, name=f"psBl{j}", tag=f"pb{j}")
                     for j in range(NVA)]
            psC_l = [psA.tile([B, VB], F32, name=f"psCl{j2}", tag=f"pa{j2}")
                     for j2 in range(NV - NVA)]
            for kt in range(KH, 16):
                t = sp.tile([128, VC], F32, tag="stream")
                nc.sync.dma_start(t[:], outwt_d[kt * 128:(kt + 1) * 128, :])
                for j in range(NVA):
                    nc.tensor.matmul(
                        psB_l[j][:], act_tile(kt), t[:, j * VB:(j + 1) * VB],
                        start=(kt == KH), stop=(kt == 15))
                for j in range(NVA, NV):
                    nc.tensor.matmul(
                        psC_l[j - NVA][:], act_tile(kt),
                        t[:, j * VB:(j + 1) * VB],
                        start=(kt == KH), stop=False)
            for j in range(NVA):
                vs = slice(j * VB, (j + 1) * VB)
                nc.vector.tensor_add(lg[:, vs], lg[:, vs], psB_l[j][:])
            for kt in range(KH):
                t = sp.tile([128, (NV - NVA) * VB], F32, tag="stream2", bufs=2)
                nc.sync.dma_start(
                    t[:], outwt_d[kt * 128:(kt + 1) * 128, NVA * VB:])
                for j in range(NVA, NV):
                    nc.tensor.matmul(
                        psC_l[j - NVA][:], act_tile(kt),
                        t[:, (j - NVA) * VB:(j - NVA + 1) * VB],
                        start=False, stop=False)
            for j in range(NVA, NV):
                vs = slice(j * VB, (j + 1) * VB)
                nc.tensor.matmul(psC_l[j - NVA][:], ones1[:, :],
                                 outb_t[:, vs], start=False, stop=True)
                nc.vector.tensor_copy(lg[:, vs], psC_l[j - NVA][:])

            # ---------- local softmax stats ----------
            st_m = wp.tile([B, 1], F32, tag="stm")
            nc.vector.reduce_max(out=st_m[:], in_=lg[:],
                                 axis=mybir.AxisListType.X)
            st_nm = wp.tile([B, 1], F32, tag="stnm")
            nc.vector.tensor_scalar_mul(st_nm[:], st_m[:], -1.0)
            st_s = wp.tile([B, 1], F32, tag="sts")
            nc.scalar.activation(scratch[:], lg[:], AF.Exp,
                                 bias=st_nm[:], accum_out=st_s[:])
            stats_t = wp.tile([B, 2], F32, tag="stats")
            nc.vector.tensor_copy(stats_t[:, 0:1], st_m[:])
            nc.vector.tensor_copy(stats_t[:, 1:2], st_s[:])

            # ---------- AG stats ----------
            b_st_in = dp.tile([B, 2], F32, tag="b_st_in")
            b_st_out = dp.tile([NC, B, 2], F32, tag="b_st_out")
            nc.scalar.dma_start(b_st_in[:], stats_t[:])
            nc.gpsimd.collective_compute(
                "AllGather", OP.bypass, replica_groups=[list(range(NC))],
                ins=[b_st_in.opt()], outs=[b_st_out.opt()])
            stall_t = wp.tile([B, 2, NC], F32, tag="stall")
            nc.scalar.dma_start(
                stall_t[:], b_st_out[:, :, :].rearrange("r b c -> b c r"))

            g_m = wp.tile([B, 1], F32, tag="gm")
            nc.vector.reduce_max(out=g_m[:], in_=stall_t[:, 0, :],
                                 axis=mybir.AxisListType.X)
            g_nm = wp.tile([B, 1], F32, tag="gnm")
            nc.vector.tensor_scalar_mul(g_nm[:], g_m[:], -1.0)
            ex_t = wp.tile([B, NC], F32, tag="ex")
            nc.scalar.activation(ex_t[:], stall_t[:, 0, :], AF.Exp, bias=g_nm[:])
            g_s = wp.tile([B, 1], F32, tag="gs")
            nc.vector.scalar_tensor_tensor(
                out=ex_t[:], in0=ex_t[:], scalar=1.0, in1=stall_t[:, 1, :],
                op0=OP.mult, op1=OP.mult, accum_out=g_s[:])
            nc.scalar.activation(g_s[:], g_s[:], AF.Ln)
            # -lse = -g_m - ln(g_s)
            g_nlse = wp.tile([B, 1], F32, tag="gnlse")
            nc.vector.tensor_sub(g_nlse[:], g_nm[:], g_s[:])
            nc.vector.tensor_scalar_add(scratch[:], lg[:], g_nlse[:])
            nc.scalar.dma_start(logp_d[:, :], scratch[:])

    nc.compile()
    return nc


_CACHE = {}


def _get_nc():
    if "nc" not in _CACHE:
        _CACHE["nc"] = _build()
    return _CACHE["nc"]


def _host_prep(inputs):
    f32 = np.float32
    ctxT = np.ascontiguousarray(np.asarray(inputs["context"], f32).T)
    h0 = np.asarray(inputs["h0"], f32)
    c0 = np.asarray(inputs["c0"], f32)
    h0T0 = np.ascontiguousarray(h0[0].T)
    h0T1 = np.ascontiguousarray(h0[1].T)
    c0T0 = np.ascontiguousarray(c0[0].T)
    c0T1 = np.ascontiguousarray(c0[1].T)
    wih0t = np.ascontiguousarray(np.asarray(inputs["W_ih0"], f32).T)
    whh0t = np.ascontiguousarray(np.asarray(inputs["W_hh0"], f32).T)
    wih1t = np.ascontiguousarray(np.asarray(inputs["W_ih1"], f32).T)
    whh1t = np.ascontiguousarray(np.asarray(inputs["W_hh1"], f32).T)
    outwt = np.ascontiguousarray(np.asarray(inputs["out_W"], f32).T)
    b0 = np.asarray(inputs["b_ih0"], f32) + np.asarray(inputs["b_hh0"], f32)
    b1 = np.asarray(inputs["b_ih1"], f32) + np.asarray(inputs["b_hh1"], f32)
    attnbt = np.ascontiguousarray(
        np.asarray(inputs["attn_b"], f32).reshape(NC, DHC).T)
    idx = np.ascontiguousarray(
        np.asarray(inputs["inputs"]).astype(np.int32).reshape(B, 1))
    emb = np.ascontiguousarray(np.asarray(inputs["emb"], f32))
    enc = np.asarray(inputs["encoder_outputs"], f32)
    outb = np.asarray(inputs["out_b"], f32)
    attnw = np.asarray(inputs["attn_W"], f32)

    in_maps = []
    for k in range(NC):
        cs = slice(k * DHC, (k + 1) * DHC)
        gcols = np.concatenate(
            [np.arange(g * DH + k * DHC, g * DH + (k + 1) * DHC)
             for g in range(4)])
        in_maps.append(dict(
            idx=idx, emb=emb, ctxT=ctxT, h0T0=h0T0, h0T1=h0T1,
            c0T0=np.ascontiguousarray(c0T0[cs]),
            c0T1=np.ascontiguousarray(c0T1[cs]),
            wih0t=np.ascontiguousarray(wih0t[:, gcols]),
            whh0t=np.ascontiguousarray(whh0t[:, gcols]),
            wih1t=np.ascontiguousarray(wih1t[:, gcols]),
            whh1t=np.ascontiguousarray(whh1t[:, gcols]),
            bias0=np.ascontiguousarray(b0[gcols].reshape(4, DHC).T),
            bias1=np.ascontiguousarray(b1[gcols].reshape(4, DHC).T),
            attnw=np.ascontiguousarray(attnw[:, cs]),
            attnbt=attnbt,
            outwt=np.ascontiguousarray(outwt[:, k * VC:(k + 1) * VC]),
            outb=np.ascontiguousarray(outb[k * VC:(k + 1) * VC].reshape(1, VC)),
            enc=np.ascontiguousarray(enc[:, :, cs]),
        ))
    return in_maps


def kernel(**inputs):
    nc = _get_nc()
    in_maps = _host_prep(inputs)
    res = bass_utils.run_bass_kernel_spmd(
        nc, in_maps, core_ids=list(range(NC)))
    r = res.results
    output = np.concatenate([r[k]["logp"] for k in range(NC)], axis=1)
    new_context = np.ascontiguousarray(r[0]["ctx_full"].T)
    h0n = np.concatenate([r[k]["h0n"] for k in range(NC)], axis=0).T
    h1n = np.concatenate([r[k]["h1n"] for k in range(NC)], axis=0).T
    c0n = np.concatenate([r[k]["c0n"] for k in range(NC)], axis=0).T
    c1n = np.concatenate([r[k]["c1n"] for k in range(NC)], axis=0).T
    h_out = np.ascontiguousarray(np.stack([h0n, h1n], axis=0))
    c_out = np.ascontiguousarray(np.stack([c0n, c1n], axis=0))
    attn_weights = np.ascontiguousarray(r[0]["logw"].T[:, None, :])
    return (output.astype(np.float32), new_context.astype(np.float32),
            h_out.astype(np.float32), c_out.astype(np.float32),
            attn_weights.astype(np.float32))


if __name__ == "__main__":
    _build()
    print("build ok")
